# revision 13
# baseline (speedup 1.0000x reference)
"""Causal self-attention Trainium2 kernel (B=4, T=4096, C=384, H=6).

Sharding: 8 cores = 4 batches x 2 head-groups (3 heads each). Each core
computes y_partial = attn(x[b], heads hg) @ w_proj[rows of hg]; the host
sums the two partials per batch (the "all-reduce after c_proj" done on
host during unshard).

v2: chunk-granular pipeline. The Scalar engine (softmax exp, ~1 elem/
cycle/lane) is the bottleneck engine; everything else (S^T matmuls,
A@V accumulation, qkv projection, c_proj, output DMA) is interleaved
into the instruction streams so ACT stays busy continuously:
 - per 128-key chunk: S^T matmul pair -> exp -> (LAG chunks later) AV
 - AV is chunk-major across 3 PSUM accumulator banks (one per head)
 - c_proj(qt-1) + phase-A(qt+1) emitted as filler inside q-tile qt
 - causal-diagonal chunks restricted to the live q-range (saves exp
   elements and PE cycles); softmax denominator rides along as a 65th
   matmul row and is transposed via 4 tiny SBUF->SBUF DMAs.
"""

import numpy as np
from contextlib import ExitStack

import concourse.bass as bass
import concourse.tile as tile
from concourse import mybir
from concourse.bass_utils import run_bass_kernel_spmd
from concourse.vector_clock import ScopedClock

F32 = mybir.dt.float32
BF16 = mybir.dt.bfloat16
EXP = mybir.ActivationFunctionType.Exp
MULT = mybir.AluOpType.mult
ADD = mybir.AluOpType.add

B, T, C, H, D = 4, 4096, 384, 6, 64
HPC = 3            # heads per core
QT = 512           # q tile
KC = 128           # key chunk
SCALE = 1.0 / 8.0  # 1/sqrt(64)
LAG = 5            # chunks between exp and AV consumption


# ---------------------------------------------------------------------------
# Workaround: neuronxcc CoreV3 rejects >2 sem waits on the Tile tail drain.
# Split the drain's waits into individual sync-engine wait instructions.
def _drain_and_barrier_split(self, tick_clock, wait_clock):
    nc = self.nc
    drain_inst = nc.sync.drain()
    wait_clock.add_sem_waits(
        drain_inst.ins, ScopedClock({None: tick_clock.global_clock})
    )
    si = drain_inst.ins.sync_info
    if si is not None and si.on_wait and len(si.on_wait) > 1:
        waits = list(si.on_wait)
        si.on_wait = []
        allocated = {h.name: h for h in self.sems.allocated().values()}
        for w in waits:
            h = allocated.get(w.ant_name)
            assert h is not None, f"no sem handle for drain wait {w.ant_name}"
            assert w.wait_mode == "sem-ge-imm", w.wait_mode
            nc.sync.wait_ge(h, w.wait_value)
    nc.all_engine_barrier()
    assert self.sems is not None
    popped = nc._tile_sem_poison_stack.pop()
    assert popped is self._sem_poison
    nc.clear_and_free_semaphores(list(self.sems.allocated().values()))
    nc.all_engine_barrier()


tile.TileContext._drain_and_barrier = _drain_and_barrier_split


MAX_WAITS = 1  # CoreV3 per-instruction sem-wait capacity (S3_LW holds only 1)


def _split_excess_waits(nc):
    """Hoist sem waits beyond MAX_WAITS onto same-engine NOPs inserted
    directly before the over-limit instruction (waits are order-free)."""
    for fn in nc.m.functions:
        for bb in fn.blocks:
            insts = list(bb.instructions)
            out = []
            changed = False
            for inst in insts:
                si = inst.sync_info
                if si is not None and si.on_wait and len(si.on_wait) > MAX_WAITS:
                    waits = list(si.on_wait)
                    excess, keep = waits[:-MAX_WAITS], waits[-MAX_WAITS:]
                    si.on_wait = keep
                    inst.sync_info = si
                    for i in range(0, len(excess), MAX_WAITS):
                        nop = mybir.InstNoOp(
                            name=f"{inst.name}-waitsplit-{i}", ins=[], outs=[]
                        )
                        nop.engine = inst.engine
                        nop.sync_info = mybir.SyncInfo(
                            on_wait=excess[i:i + MAX_WAITS], on_update=[]
                        )
                        nc.register_instruction(nop)
                        out.append(nop)
                    changed = True
                out.append(inst)
            if changed:
                bb.instructions = out
# ---------------------------------------------------------------------------


def build(t=T):
    nqt = t // QT          # q tiles
    nkc = t // KC          # key chunks

    nc = bass.Bass()
    import itertools
    _dmaq = itertools.cycle([nc.sync, nc.gpsimd])

    def dma(out, in_):
        next(_dmaq).dma_start(out=out, in_=in_)
    x_d = nc.dram_tensor("xT16", [C, t], BF16, kind="ExternalInput")
    wq01_d = nc.dram_tensor("w_q01", [3, 128, 128], BF16, kind="ExternalInput")
    wk01_d = nc.dram_tensor("w_k01", [3, 128, 128], BF16, kind="ExternalInput")
    wq2_d = nc.dram_tensor("w_q2", [3, 128, 64], BF16, kind="ExternalInput")
    wk2_d = nc.dram_tensor("w_k2", [3, 128, 64], BF16, kind="ExternalInput")
    wv_d = nc.dram_tensor("w_v", [3, 128, 192], BF16, kind="ExternalInput")
    wo_d = nc.dram_tensor("w_o", [3, 64, 384], BF16, kind="ExternalInput")
    mask_d = nc.dram_tensor("masks", [128, 128], BF16, kind="ExternalInput")
    y_d = nc.dram_tensor("y", [t, C], F32, kind="ExternalOutput")

    with tile.TileContext(nc) as tc, ExitStack() as ctx:
        persist = ctx.enter_context(tc.tile_pool(name="persist", bufs=1))

        # weights / mask (DMAs emitted in the prologue, round-robin queues)
        wq01 = persist.tile([128, 3, 128], BF16)
        wk01 = persist.tile([128, 3, 128], BF16)
        wq2 = persist.tile([128, 3, 64], BF16)
        wk2 = persist.tile([128, 3, 64], BF16)
        wv = persist.tile([128, 3, 192], BF16)
        wo = persist.tile([64, 3, 384], BF16)
        m128 = persist.tile([128, 128], BF16)

        # persistent activations (bf16)
        qT01 = persist.tile([128, t], BF16)   # rows 0:64 h0 qT, 64:128 h1 qT
        kT01 = persist.tile([128, t], BF16)
        # head 2 q/k duplicated into both partition halves so chunk pairs
        # can run as concurrent row-group-packed matmuls
        qT2 = persist.tile([128, t], BF16)
        kT2 = persist.tile([128, t], BF16)
        vsb = persist.tile([128, nkc, 3, 65], BF16)  # [keys, chunk, head, d|one]
        nc.vector.memset(vsb[:, :, :, 64:65], 1.0)

        with (
            tc.tile_pool(name="xt", bufs=3) as xt_p,
            tc.tile_pool(name="ps", bufs=2, space="PSUM") as ps_p,        # 2x 2 banks
            tc.tile_pool(name="ps_att", bufs=1, space="PSUM") as psatt_p,  # 3 banks
            tc.tile_pool(name="ps_y", bufs=1, space="PSUM") as psy_p,      # 1 bank
            tc.tile_pool(name="pth01", bufs=10) as pth01_p,
            tc.tile_pool(name="pth2", bufs=6) as pth2_p,
            tc.tile_pool(name="at", bufs=6) as at_p,
            tc.tile_pool(name="lin", bufs=12) as lin_p,
            tc.tile_pool(name="yout", bufs=3) as yout_p,
        ):
            xts = {}        # tb -> xT tile
            pth01s = {}     # (qt, ck) -> tile
            pth2s = {}      # (qt, g) -> tile
            atts = {}       # qt -> psum att tile [65, 3, 512]
            ats = {}        # qt -> [at_h tiles]
            linvs = {}      # qt -> [linv tiles]

            def load_x(tb):
                xT = xt_p.tile([128, 3, QT], BF16, tag="xt", name="xT")
                xts[tb] = xT
                for c in range(3):
                    dma(
                        xT[:, c, :],
                        x_d[c * 128:(c + 1) * 128, tb * QT:(tb + 1) * QT],
                    )

            def dummy_mm():
                # keeps the PE HAM activity monitor busy so the clock gate
                # stays at K=8/8 (2.4 GHz); result is never read
                yp = psy_p.tile([128, C], F32, tag="y", name="ydum")
                nc.tensor.matmul(
                    yp[:], wq01[:, 0, :],
                    wq01[:, :, :].rearrange("p c m -> p (c m)"),
                    start=True, stop=True,
                )

            def a_qk(tb, gi):
                w_sb, m, dst = (
                    (wq01, 128, qT01), (wk01, 128, kT01),
                    (wq2, 64, qT2), (wk2, 64, kT2),
                )[gi]
                xT = xts[tb]
                ps = ps_p.tile([128, QT], F32, tag="ps", name="psqk")
                for c in range(3):
                    nc.tensor.matmul(
                        ps[0:m, :], w_sb[:, c, 0:m], xT[:, c, :],
                        start=(c == 0), stop=(c == 2),
                    )
                nc.vector.tensor_copy(
                    dst[0:m, tb * QT:(tb + 1) * QT], ps[0:m, :]
                )

            def a_dup(tb):
                dma(
                    qT2[64:128, tb * QT:(tb + 1) * QT],
                    qT2[0:64, tb * QT:(tb + 1) * QT],
                )
                dma(
                    kT2[64:128, tb * QT:(tb + 1) * QT],
                    kT2[0:64, tb * QT:(tb + 1) * QT],
                )

            def a_v(tb, s):
                xT = xts[tb]
                psv = ps_p.tile([128, 3, 64], F32, tag="ps", name="psv")
                for c in range(3):
                    nc.tensor.matmul(
                        psv[:, :, :].rearrange("p h d -> p (h d)"),
                        xT[:, c, s * 128:(s + 1) * 128],
                        wv[:, c, :],
                        start=(c == 0), stop=(c == 2),
                    )
                nc.vector.tensor_copy(
                    vsb[:, tb * 4 + s, :, 0:64], psv[:, :, :]
                )

            def emit_unit(qt, ck):
                """S^T + exp (+ masks) for chunk ck of q tile qt."""
                q0, q1 = qt * QT, (qt + 1) * QT
                diag = ck >= 4 * qt
                qlo = 128 * (ck - 4 * qt) if diag else 0

                # h0/h1: row-group packed pair into one 2-bank psum tile
                ssx = ps_p.tile([128, 2, QT], F32, tag="ps", name="ssx")
                nc.tensor.matmul(
                    ssx[:, 0, qlo:],
                    kT01[0:64, ck * KC:(ck + 1) * KC],
                    qT01[0:64, q0 + qlo:q1],
                    start=True, stop=True, tile_position=(0, 0),
                )
                nc.tensor.matmul(
                    ssx[:, 1, qlo:],
                    kT01[64:128, ck * KC:(ck + 1) * KC],
                    qT01[64:128, q0 + qlo:q1],
                    start=True, stop=True, tile_position=(64, 0),
                )
                p01 = pth01_p.tile([128, 2, QT], BF16, tag="pth01", name="p01")
                pth01s[(qt, ck)] = p01
                nc.scalar.activation(
                    out=p01[:, :, qlo:], in_=ssx[:, :, qlo:],
                    func=EXP, scale=SCALE,
                )

                # h2: two chunks (2g, 2g+1) packed as concurrent row groups
                if ck % 2 == 0:
                    g = ck // 2
                    ck1 = ck + 1
                    diag1 = ck1 >= 4 * qt
                    qlo1 = 128 * (ck1 - 4 * qt) if diag1 else 0
                    ssc = ps_p.tile([128, 2, QT], F32, tag="ps", name="ssc")
                    nc.tensor.matmul(
                        ssc[:, 0, qlo:],
                        kT2[0:64, ck * KC:(ck + 1) * KC],
                        qT2[0:64, q0 + qlo:q1],
                        start=True, stop=True, tile_position=(0, 0),
                    )
                    nc.tensor.matmul(
                        ssc[:, 1, qlo1:],
                        kT2[64:128, ck1 * KC:(ck1 + 1) * KC],
                        qT2[64:128, q0 + qlo1:q1],
                        start=True, stop=True, tile_position=(64, 0),
                    )
                    p2 = pth2_p.tile([128, 2, QT], BF16, tag="pth2", name="p2")
                    pth2s[(qt, g)] = p2
                    if qlo == qlo1:
                        nc.scalar.activation(
                            out=p2[:, :, qlo:], in_=ssc[:, :, qlo:],
                            func=EXP, scale=SCALE,
                        )
                    else:
                        nc.scalar.activation(
                            out=p2[:, 0, qlo:], in_=ssc[:, 0, qlo:],
                            func=EXP, scale=SCALE,
                        )
                        nc.scalar.activation(
                            out=p2[:, 1, qlo1:], in_=ssc[:, 1, qlo1:],
                            func=EXP, scale=SCALE,
                        )

                # causal mask on the diagonal 128-col window
                if diag:
                    w0, w1 = qlo, qlo + 128
                    for h in range(2):
                        nc.vector.tensor_tensor(
                            out=p01[:, h, w0:w1], in0=p01[:, h, w0:w1],
                            in1=m128[:], op=MULT,
                        )
                    p2t = pth2s[(qt, ck // 2)]
                    nc.vector.tensor_tensor(
                        out=p2t[:, ck % 2, w0:w1], in0=p2t[:, ck % 2, w0:w1],
                        in1=m128[:], op=MULT,
                    )

            def get_att(qt):
                if qt not in atts:
                    atts[qt] = psatt_p.tile(
                        [65, 3, QT], F32, tag="att", name="att"
                    )
                return atts[qt]

            def emit_av(qt, ck):
                nch = 4 * (qt + 1)
                att = get_att(qt)
                diag = ck >= 4 * qt
                qlo = 128 * (ck - 4 * qt) if diag else 0
                first, last = ck == 0, ck == nch - 1
                for h in range(3):
                    if h < 2:
                        rhs = pth01s[(qt, ck)][:, h, qlo:]
                    else:
                        rhs = pth2s[(qt, ck // 2)][:, ck % 2, qlo:]
                    nc.tensor.matmul(
                        att[:, h, qlo:], vsb[:, ck, h, :], rhs,
                        start=first, stop=last, skip_group_check=True,
                    )

            def emit_epilogue(qt):
                """att psum -> at (bf16, incl. denom row); l -> linv."""
                att = atts[qt]
                ats[qt] = []
                linvs[qt] = []
                for h in range(3):
                    at = at_p.tile([65, QT], BF16, tag="at", name="at")
                    ats[qt].append(at)
                    nc.vector.tensor_copy(at[:], att[:, h, :])
                    lcol = lin_p.tile([128, 4], BF16, tag="lcol", name="lcol")
                    for s in range(4):
                        dma(
                            lcol[:, s:s + 1],
                            at[64:65, s * 128:(s + 1) * 128],
                        )
                    linv = lin_p.tile([128, 4], F32, tag="linv", name="linv")
                    linvs[qt].append(linv)
                    nc.vector.reciprocal(linv[:], lcol[:])

            def emit_cproj_item(qt, s, h):
                """One head's c_proj matmul + normalize-accumulate for
                s-block s of q tile qt; DMA out after h==2."""
                q0 = qt * QT
                at = ats[qt][h]
                linv = linvs[qt][h]
                if h == 0:
                    ysb = yout_p.tile([128, C], F32, tag="ysb", name="ysb")
                    emit_cproj_item.ysb = ysb
                else:
                    ysb = emit_cproj_item.ysb
                yp = psy_p.tile([128, C], F32, tag="y", name="yp")
                nc.tensor.matmul(
                    yp[:], at[0:64, s * 128:(s + 1) * 128], wo[:, h, :],
                    start=True, stop=True,
                )
                sc = linv[:, s:s + 1]
                if h == 0:
                    nc.vector.tensor_scalar(
                        out=ysb[:], in0=yp[:], scalar1=sc,
                        scalar2=None, op0=MULT,
                    )
                else:
                    nc.vector.scalar_tensor_tensor(
                        out=ysb[:], in0=yp[:], scalar=sc, in1=ysb[:],
                        op0=MULT, op1=ADD,
                    )
                if h == 2:
                    nc.gpsimd.dma_start(
                        out=y_d[q0 + s * 128:q0 + (s + 1) * 128, :],
                        in_=ysb[:],
                    )

            # ---------------- prologue ----------------
            load_x(0)
            for c in range(3):
                dma(wq01[:, c, :], wq01_d[c])
                dma(wk01[:, c, :], wk01_d[c])
                dma(wq2[:, c, :], wq2_d[c])
                dma(wk2[:, c, :], wk2_d[c])
            load_x(1)
            for c in range(3):
                dma(wv[:, c, :], wv_d[c])
                dma(wo[:, c, :], wo_d[c])
            dma(m128[:], mask_d[:])
            for gi in range(4):
                a_qk(0, gi)
            a_dup(0)
            for s in range(4):
                a_v(0, s)

            # ---------------- main loop ----------------
            pending_av = []   # (qt, ck) AV triples carried into next qt
            lazy_q = []       # c_proj items; may straggle across qt bounds

            for qt in range(nqt):
                nch = 4 * (qt + 1)

                main_q = []
                if qt + 2 < nqt:
                    main_q.append(lambda tb=qt + 2: load_x(tb))
                carried, pending_av = pending_av, []
                for (pqt, pck) in carried:
                    main_q.append(lambda a=pqt, b=pck: emit_av(a, b))
                if qt >= 1:
                    main_q.append(lambda a=qt - 1: emit_epilogue(a))
                if qt + 1 < nqt:
                    for gi in range(4):
                        main_q.append(lambda tb=qt + 1, g=gi: a_qk(tb, g))
                    main_q.append(lambda tb=qt + 1: a_dup(tb))
                    for s in range(4):
                        main_q.append(lambda tb=qt + 1, b=s: a_v(tb, b))
                if qt >= 1:
                    for s in range(4):
                        for h in range(3):
                            lazy_q.append(
                                lambda a=qt - 1, b=s, c=h:
                                emit_cproj_item(a, b, c)
                            )

                mi = 0
                for ck in range(nch):
                    emit_unit(qt, ck)
                    if ck >= LAG:
                        emit_av(qt, ck - LAG)
                    for _ in range(2):
                        if mi < len(main_q):
                            main_q[mi]()
                            mi += 1
                        elif lazy_q:
                            lazy_q.pop(0)()
                    dummy_mm()
                while mi < len(main_q):
                    main_q[mi]()
                    mi += 1

                for ck in range(max(0, nch - LAG), nch):
                    pending_av.append((qt, ck))

            # ---------------- tail: qt = nqt-1, per-s pipelined ----------
            lq = nqt - 1
            while lazy_q:
                lazy_q.pop(0)()

            def tail_cproj_s(s):
                """at/linv extraction + c_proj for s-block s of last tile."""
                att = atts[lq]
                q0 = lq * QT
                ysb = yout_p.tile([128, C], F32, tag="ysb", name="ysb")
                for h in range(3):
                    at = at_p.tile([65, 128], BF16, tag="at", name="at_s")
                    nc.vector.tensor_copy(
                        at[:], att[:, h, s * 128:(s + 1) * 128]
                    )
                    lcol = lin_p.tile([128, 1], BF16, tag="lcol", name="lcol")
                    dma(lcol[:, :], at[64:65, :])
                    linv = lin_p.tile([128, 1], F32, tag="linv", name="linv")
                    nc.vector.reciprocal(linv[:], lcol[:])
                    yp = psy_p.tile([128, C], F32, tag="y", name="yp")
                    nc.tensor.matmul(
                        yp[:], at[0:64, :], wo[:, h, :],
                        start=True, stop=True,
                    )
                    if h == 0:
                        nc.vector.tensor_scalar(
                            out=ysb[:], in0=yp[:], scalar1=linv[:, 0:1],
                            scalar2=None, op0=MULT,
                        )
                    else:
                        nc.vector.scalar_tensor_tensor(
                            out=ysb[:], in0=yp[:], scalar=linv[:, 0:1],
                            in1=ysb[:], op0=MULT, op1=ADD,
                        )
                    dummy_mm()
                nc.gpsimd.dma_start(
                    out=y_d[q0 + s * 128:q0 + (s + 1) * 128, :], in_=ysb[:],
                )

            nch = 4 * nqt
            for (pqt, pck) in pending_av:
                emit_av(pqt, pck)
                dummy_mm()
                # s-block s of the last q tile is final once the diagonal
                # chunk with qlo = 128*s has accumulated (ck = nch-4+s)
                if pck >= nch - 4:
                    tail_cproj_s(pck - (nch - 4))

    _split_excess_waits(nc)
    nc.finalize()
    return nc


_NC_CACHE = {}


def _get_nc(t=T):
    if t not in _NC_CACHE:
        _NC_CACHE[t] = build(t)
    return _NC_CACHE[t]


def _prep_core_inputs(x_b, w_attn, w_proj, hg, bf16):
    """Host-side shard prep for one core: batch x_b, head group hg (0/1)."""
    h0 = 3 * hg
    q = w_attn[:, 0:C]
    k = w_attn[:, C:2 * C]
    v = w_attn[:, 2 * C:3 * C]
    qcols = lambda h: q[:, h * D:(h + 1) * D]
    kcols = lambda h: k[:, h * D:(h + 1) * D]
    w_q01 = np.concatenate([qcols(h0), qcols(h0 + 1)], axis=1)      # [384,128]
    w_k01 = np.concatenate([kcols(h0), kcols(h0 + 1)], axis=1)
    w_q2 = qcols(h0 + 2)                                            # [384,64]
    w_k2 = kcols(h0 + 2)
    w_v = v[:, h0 * D:(h0 + 3) * D]                                 # [384,192]
    w_o = w_proj[h0 * D:(h0 + 3) * D, :]                            # [192,384]
    return {
        "xT16": np.ascontiguousarray(x_b.T, dtype=bf16),
        "w_q01": np.ascontiguousarray(w_q01.reshape(3, 128, 128), dtype=bf16),
        "w_k01": np.ascontiguousarray(w_k01.reshape(3, 128, 128), dtype=bf16),
        "w_q2": np.ascontiguousarray(w_q2.reshape(3, 128, 64), dtype=bf16),
        "w_k2": np.ascontiguousarray(w_k2.reshape(3, 128, 64), dtype=bf16),
        "w_v": np.ascontiguousarray(w_v.reshape(3, 128, 192), dtype=bf16),
        "w_o": np.ascontiguousarray(w_o.reshape(3, 64, 384), dtype=bf16),
    }


def _make_masks(bf16):
    f = np.arange(128)[None, :]
    p = np.arange(128)[:, None]
    return (f >= p).astype(np.float32).astype(bf16)


def kernel(x, w_attn, w_proj):
    import ml_dtypes
    bf16 = ml_dtypes.bfloat16

    x = np.asarray(x, dtype=np.float32)
    w_attn = np.asarray(w_attn, dtype=np.float32)
    w_proj = np.asarray(w_proj, dtype=np.float32)
    b, t, c = x.shape

    nc = _get_nc(t)
    masks = _make_masks(bf16)
    in_maps = []
    for core in range(8):
        im = _prep_core_inputs(x[core // 2], w_attn, w_proj, core % 2, bf16)
        im["masks"] = masks
        in_maps.append(im)

    res = run_bass_kernel_spmd(nc, in_maps, list(range(8)))
    out = np.empty((b, t, c), dtype=np.float32)
    for bb in range(b):
        out[bb] = res.results[2 * bb]["y"] + res.results[2 * bb + 1]["y"]
    return out


# revision 17
# speedup vs baseline: 1.0119x; 1.0119x over previous
"""Causal self-attention Trainium2 kernel (B=4, T=4096, C=384, H=6).

Sharding: 8 cores = 4 batches x 2 head-groups (3 heads each). Each core
computes y_partial = attn(x[b], heads hg) @ w_proj[rows of hg]; the host
sums the two partials per batch (the "all-reduce after c_proj" done on
host during unshard).

v2: chunk-granular pipeline. The Scalar engine (softmax exp, ~1 elem/
cycle/lane) is the bottleneck engine; everything else (S^T matmuls,
A@V accumulation, qkv projection, c_proj, output DMA) is interleaved
into the instruction streams so ACT stays busy continuously:
 - per 128-key chunk: S^T matmul pair -> exp -> (LAG chunks later) AV
 - AV is chunk-major across 3 PSUM accumulator banks (one per head)
 - c_proj(qt-1) + phase-A(qt+1) emitted as filler inside q-tile qt
 - causal-diagonal chunks restricted to the live q-range (saves exp
   elements and PE cycles); softmax denominator rides along as a 65th
   matmul row and is transposed via 4 tiny SBUF->SBUF DMAs.
"""

import numpy as np
from contextlib import ExitStack

import concourse.bass as bass
import concourse.tile as tile
from concourse import mybir
from concourse.bass_utils import run_bass_kernel_spmd
from concourse.vector_clock import ScopedClock

F32 = mybir.dt.float32
BF16 = mybir.dt.bfloat16
EXP = mybir.ActivationFunctionType.Exp
MULT = mybir.AluOpType.mult
ADD = mybir.AluOpType.add

B, T, C, H, D = 4, 4096, 384, 6, 64
HPC = 3            # heads per core
QT = 512           # q tile
KC = 128           # key chunk
SCALE = 1.0 / 8.0  # 1/sqrt(64)
LAG = 5            # chunks between exp and AV consumption


# ---------------------------------------------------------------------------
# Workaround: neuronxcc CoreV3 rejects >2 sem waits on the Tile tail drain.
# Split the drain's waits into individual sync-engine wait instructions.
def _drain_and_barrier_split(self, tick_clock, wait_clock):
    nc = self.nc
    drain_inst = nc.sync.drain()
    wait_clock.add_sem_waits(
        drain_inst.ins, ScopedClock({None: tick_clock.global_clock})
    )
    si = drain_inst.ins.sync_info
    if si is not None and si.on_wait and len(si.on_wait) > 1:
        waits = list(si.on_wait)
        si.on_wait = []
        allocated = {h.name: h for h in self.sems.allocated().values()}
        for w in waits:
            h = allocated.get(w.ant_name)
            assert h is not None, f"no sem handle for drain wait {w.ant_name}"
            assert w.wait_mode == "sem-ge-imm", w.wait_mode
            nc.sync.wait_ge(h, w.wait_value)
    nc.all_engine_barrier()
    assert self.sems is not None
    popped = nc._tile_sem_poison_stack.pop()
    assert popped is self._sem_poison
    nc.clear_and_free_semaphores(list(self.sems.allocated().values()))
    nc.all_engine_barrier()


tile.TileContext._drain_and_barrier = _drain_and_barrier_split


MAX_WAITS = 1  # CoreV3 per-instruction sem-wait capacity (S3_LW holds only 1)


def _split_excess_waits(nc):
    """Hoist sem waits beyond MAX_WAITS onto same-engine NOPs inserted
    directly before the over-limit instruction (waits are order-free)."""
    for fn in nc.m.functions:
        for bb in fn.blocks:
            insts = list(bb.instructions)
            out = []
            changed = False
            for inst in insts:
                si = inst.sync_info
                if si is not None and si.on_wait and len(si.on_wait) > MAX_WAITS:
                    waits = list(si.on_wait)
                    excess, keep = waits[:-MAX_WAITS], waits[-MAX_WAITS:]
                    si.on_wait = keep
                    inst.sync_info = si
                    for i in range(0, len(excess), MAX_WAITS):
                        nop = mybir.InstNoOp(
                            name=f"{inst.name}-waitsplit-{i}", ins=[], outs=[]
                        )
                        nop.engine = inst.engine
                        nop.sync_info = mybir.SyncInfo(
                            on_wait=excess[i:i + MAX_WAITS], on_update=[]
                        )
                        nc.register_instruction(nop)
                        out.append(nop)
                    changed = True
                out.append(inst)
            if changed:
                bb.instructions = out
# ---------------------------------------------------------------------------


def build(t=T):
    nqt = t // QT          # q tiles
    nkc = t // KC          # key chunks

    nc = bass.Bass()
    import itertools
    _dmaq = itertools.cycle([nc.sync, nc.gpsimd])

    def dma(out, in_):
        next(_dmaq).dma_start(out=out, in_=in_)
    x_d = nc.dram_tensor("xT16", [C, t], BF16, kind="ExternalInput")
    wq01_d = nc.dram_tensor("w_q01", [3, 128, 128], BF16, kind="ExternalInput")
    wk01_d = nc.dram_tensor("w_k01", [3, 128, 128], BF16, kind="ExternalInput")
    wq2_d = nc.dram_tensor("w_q2", [3, 128, 64], BF16, kind="ExternalInput")
    wk2_d = nc.dram_tensor("w_k2", [3, 128, 64], BF16, kind="ExternalInput")
    wv_d = nc.dram_tensor("w_v", [3, 128, 192], BF16, kind="ExternalInput")
    wo_d = nc.dram_tensor("w_o", [3, 64, 384], BF16, kind="ExternalInput")
    mask_d = nc.dram_tensor("masks", [128, 128], BF16, kind="ExternalInput")
    y_d = nc.dram_tensor("y", [t, C], F32, kind="ExternalOutput")

    with tile.TileContext(nc) as tc, ExitStack() as ctx:
        persist = ctx.enter_context(tc.tile_pool(name="persist", bufs=1))

        # weights / mask (DMAs emitted in the prologue, round-robin queues)
        wq01 = persist.tile([128, 3, 128], BF16)
        wk01 = persist.tile([128, 3, 128], BF16)
        wq2 = persist.tile([128, 3, 64], BF16)
        wk2 = persist.tile([128, 3, 64], BF16)
        wv = persist.tile([128, 3, 192], BF16)
        wo = persist.tile([64, 3, 384], BF16)
        m128 = persist.tile([128, 128], BF16)

        # persistent activations (bf16)
        qT01 = persist.tile([128, t], BF16)   # rows 0:64 h0 qT, 64:128 h1 qT
        kT01 = persist.tile([128, t], BF16)
        # head 2 q/k duplicated into both partition halves so chunk pairs
        # can run as concurrent row-group-packed matmuls
        qT2 = persist.tile([128, t], BF16)
        kT2 = persist.tile([128, t], BF16)
        vsb = persist.tile([128, nkc, 3, 65], BF16)  # [keys, chunk, head, d|one]
        nc.vector.memset(vsb[:, :, :, 64:65], 1.0)

        with (
            tc.tile_pool(name="xt", bufs=3) as xt_p,
            tc.tile_pool(name="ps", bufs=2, space="PSUM") as ps_p,        # 2x 2 banks
            tc.tile_pool(name="ps_att", bufs=1, space="PSUM") as psatt_p,  # 3 banks
            tc.tile_pool(name="ps_y", bufs=1, space="PSUM") as psy_p,      # 1 bank
            tc.tile_pool(name="pth01", bufs=10) as pth01_p,
            tc.tile_pool(name="pth2", bufs=6) as pth2_p,
            tc.tile_pool(name="at", bufs=6) as at_p,
            tc.tile_pool(name="lin", bufs=12) as lin_p,
            tc.tile_pool(name="yout", bufs=3) as yout_p,
        ):
            xts = {}        # tb -> xT tile
            pth01s = {}     # (qt, ck) -> tile
            pth2s = {}      # (qt, g) -> tile
            atts = {}       # qt -> psum att tile [65, 3, 512]
            ats = {}        # qt -> [at_h tiles]
            linvs = {}      # qt -> [linv tiles]

            def load_x(tb):
                xT = xt_p.tile([128, 3, QT], BF16, tag="xt", name="xT")
                xts[tb] = xT
                for c in range(3):
                    dma(
                        xT[:, c, :],
                        x_d[c * 128:(c + 1) * 128, tb * QT:(tb + 1) * QT],
                    )

            def warm_burst(n=12):
                # The PE HAM clock gate only flips to K=8/8 (2.4 GHz) after a
                # fully-busy 4096-cycle window, and once warm it stays warm
                # through the pipeline's sub-microsecond gaps. Force the flip
                # with a dense dependency-free matmul burst; result unused.
                wb = ps_p.tile([128, 2, QT], F32, tag="ps", name="warmb")
                for _ in range(n):
                    nc.tensor.matmul(
                        wb[:, 0, 0:384], wq01[:, 0, :],
                        wq01[:, :, :].rearrange("p c m -> p (c m)"),
                        start=True, stop=True,
                    )

            def a_qk(tb, gi):
                w_sb, m, dst = (
                    (wq01, 128, qT01), (wk01, 128, kT01),
                    (wq2, 64, qT2), (wk2, 64, kT2),
                )[gi]
                xT = xts[tb]
                ps = ps_p.tile([128, QT], F32, tag="ps", name="psqk")
                for c in range(3):
                    nc.tensor.matmul(
                        ps[0:m, :], w_sb[:, c, 0:m], xT[:, c, :],
                        start=(c == 0), stop=(c == 2),
                    )
                nc.vector.tensor_copy(
                    dst[0:m, tb * QT:(tb + 1) * QT], ps[0:m, :]
                )

            def a_dup(tb):
                dma(
                    qT2[64:128, tb * QT:(tb + 1) * QT],
                    qT2[0:64, tb * QT:(tb + 1) * QT],
                )
                dma(
                    kT2[64:128, tb * QT:(tb + 1) * QT],
                    kT2[0:64, tb * QT:(tb + 1) * QT],
                )

            def a_v(tb, s):
                xT = xts[tb]
                psv = ps_p.tile([128, 3, 64], F32, tag="ps", name="psv")
                for c in range(3):
                    nc.tensor.matmul(
                        psv[:, :, :].rearrange("p h d -> p (h d)"),
                        xT[:, c, s * 128:(s + 1) * 128],
                        wv[:, c, :],
                        start=(c == 0), stop=(c == 2),
                    )
                nc.vector.tensor_copy(
                    vsb[:, tb * 4 + s, :, 0:64], psv[:, :, :]
                )

            def emit_unit(qt, ck):
                """S^T + exp (+ masks) for chunk ck of q tile qt."""
                q0, q1 = qt * QT, (qt + 1) * QT
                diag = ck >= 4 * qt
                qlo = 128 * (ck - 4 * qt) if diag else 0

                # h0/h1: row-group packed pair into one 2-bank psum tile
                ssx = ps_p.tile([128, 2, QT], F32, tag="ps", name="ssx")
                nc.tensor.matmul(
                    ssx[:, 0, qlo:],
                    kT01[0:64, ck * KC:(ck + 1) * KC],
                    qT01[0:64, q0 + qlo:q1],
                    start=True, stop=True, tile_position=(0, 0),
                )
                nc.tensor.matmul(
                    ssx[:, 1, qlo:],
                    kT01[64:128, ck * KC:(ck + 1) * KC],
                    qT01[64:128, q0 + qlo:q1],
                    start=True, stop=True, tile_position=(64, 0),
                )
                p01 = pth01_p.tile([128, 2, QT], BF16, tag="pth01", name="p01")
                pth01s[(qt, ck)] = p01
                nc.scalar.activation(
                    out=p01[:, :, qlo:], in_=ssx[:, :, qlo:],
                    func=EXP, scale=SCALE,
                )

                # h2: two chunks (2g, 2g+1) packed as concurrent row groups
                if ck % 2 == 0:
                    g = ck // 2
                    ck1 = ck + 1
                    diag1 = ck1 >= 4 * qt
                    qlo1 = 128 * (ck1 - 4 * qt) if diag1 else 0
                    ssc = ps_p.tile([128, 2, QT], F32, tag="ps", name="ssc")
                    nc.tensor.matmul(
                        ssc[:, 0, qlo:],
                        kT2[0:64, ck * KC:(ck + 1) * KC],
                        qT2[0:64, q0 + qlo:q1],
                        start=True, stop=True, tile_position=(0, 0),
                    )
                    nc.tensor.matmul(
                        ssc[:, 1, qlo1:],
                        kT2[64:128, ck1 * KC:(ck1 + 1) * KC],
                        qT2[64:128, q0 + qlo1:q1],
                        start=True, stop=True, tile_position=(64, 0),
                    )
                    p2 = pth2_p.tile([128, 2, QT], BF16, tag="pth2", name="p2")
                    pth2s[(qt, g)] = p2
                    if qlo == qlo1:
                        nc.scalar.activation(
                            out=p2[:, :, qlo:], in_=ssc[:, :, qlo:],
                            func=EXP, scale=SCALE,
                        )
                    else:
                        nc.scalar.activation(
                            out=p2[:, 0, qlo:], in_=ssc[:, 0, qlo:],
                            func=EXP, scale=SCALE,
                        )
                        nc.scalar.activation(
                            out=p2[:, 1, qlo1:], in_=ssc[:, 1, qlo1:],
                            func=EXP, scale=SCALE,
                        )

                # causal mask on the diagonal 128-col window
                if diag:
                    w0, w1 = qlo, qlo + 128
                    for h in range(2):
                        nc.vector.tensor_tensor(
                            out=p01[:, h, w0:w1], in0=p01[:, h, w0:w1],
                            in1=m128[:], op=MULT,
                        )
                    p2t = pth2s[(qt, ck // 2)]
                    nc.vector.tensor_tensor(
                        out=p2t[:, ck % 2, w0:w1], in0=p2t[:, ck % 2, w0:w1],
                        in1=m128[:], op=MULT,
                    )

            def get_att(qt):
                if qt not in atts:
                    atts[qt] = psatt_p.tile(
                        [65, 3, QT], F32, tag="att", name="att"
                    )
                return atts[qt]

            def emit_av(qt, ck):
                nch = 4 * (qt + 1)
                att = get_att(qt)
                diag = ck >= 4 * qt
                qlo = 128 * (ck - 4 * qt) if diag else 0
                first, last = ck == 0, ck == nch - 1
                for h in range(3):
                    if h < 2:
                        rhs = pth01s[(qt, ck)][:, h, qlo:]
                    else:
                        rhs = pth2s[(qt, ck // 2)][:, ck % 2, qlo:]
                    nc.tensor.matmul(
                        att[:, h, qlo:], vsb[:, ck, h, :], rhs,
                        start=first, stop=last, skip_group_check=True,
                    )

            def emit_epilogue(qt):
                """att psum -> at (bf16, incl. denom row); l -> linv."""
                att = atts[qt]
                ats[qt] = []
                linvs[qt] = []
                for h in range(3):
                    at = at_p.tile([65, QT], BF16, tag="at", name="at")
                    ats[qt].append(at)
                    nc.vector.tensor_copy(at[:], att[:, h, :])
                    lcol = lin_p.tile([128, 4], BF16, tag="lcol", name="lcol")
                    for s in range(4):
                        dma(
                            lcol[:, s:s + 1],
                            at[64:65, s * 128:(s + 1) * 128],
                        )
                    linv = lin_p.tile([128, 4], F32, tag="linv", name="linv")
                    linvs[qt].append(linv)
                    nc.vector.reciprocal(linv[:], lcol[:])

            def emit_cproj_item(qt, s, h):
                """One head's c_proj matmul + normalize-accumulate for
                s-block s of q tile qt; DMA out after h==2."""
                q0 = qt * QT
                at = ats[qt][h]
                linv = linvs[qt][h]
                if h == 0:
                    ysb = yout_p.tile([128, C], F32, tag="ysb", name="ysb")
                    emit_cproj_item.ysb = ysb
                else:
                    ysb = emit_cproj_item.ysb
                yp = psy_p.tile([128, C], F32, tag="y", name="yp")
                nc.tensor.matmul(
                    yp[:], at[0:64, s * 128:(s + 1) * 128], wo[:, h, :],
                    start=True, stop=True,
                )
                sc = linv[:, s:s + 1]
                if h == 0:
                    nc.vector.tensor_scalar(
                        out=ysb[:], in0=yp[:], scalar1=sc,
                        scalar2=None, op0=MULT,
                    )
                else:
                    nc.vector.scalar_tensor_tensor(
                        out=ysb[:], in0=yp[:], scalar=sc, in1=ysb[:],
                        op0=MULT, op1=ADD,
                    )
                if h == 2:
                    nc.gpsimd.dma_start(
                        out=y_d[q0 + s * 128:q0 + (s + 1) * 128, :],
                        in_=ysb[:],
                    )

            # ---------------- prologue ----------------
            load_x(0)
            for c in range(3):
                dma(wq01[:, c, :], wq01_d[c])
                dma(wk01[:, c, :], wk01_d[c])
                dma(wq2[:, c, :], wq2_d[c])
                dma(wk2[:, c, :], wk2_d[c])
            load_x(1)
            for c in range(3):
                dma(wv[:, c, :], wv_d[c])
                dma(wo[:, c, :], wo_d[c])
            dma(m128[:], mask_d[:])
            warm_burst()
            for gi in range(4):
                a_qk(0, gi)
            a_dup(0)
            for s in range(4):
                a_v(0, s)

            # ---------------- main loop ----------------
            pending_av = []   # (qt, ck) AV triples carried into next qt
            lazy_q = []       # c_proj items; may straggle across qt bounds

            for qt in range(nqt):
                nch = 4 * (qt + 1)

                main_q = []
                if qt + 2 < nqt:
                    main_q.append(lambda tb=qt + 2: load_x(tb))
                carried, pending_av = pending_av, []
                for (pqt, pck) in carried:
                    main_q.append(lambda a=pqt, b=pck: emit_av(a, b))
                if qt >= 1:
                    main_q.append(lambda a=qt - 1: emit_epilogue(a))
                if qt + 1 < nqt:
                    for gi in range(4):
                        main_q.append(lambda tb=qt + 1, g=gi: a_qk(tb, g))
                    main_q.append(lambda tb=qt + 1: a_dup(tb))
                    for s in range(4):
                        main_q.append(lambda tb=qt + 1, b=s: a_v(tb, b))
                if qt >= 1:
                    for s in range(4):
                        for h in range(3):
                            lazy_q.append(
                                lambda a=qt - 1, b=s, c=h:
                                emit_cproj_item(a, b, c)
                            )

                if qt >= 1:
                    warm_burst()
                mi = 0
                for ck in range(nch):
                    emit_unit(qt, ck)
                    if ck >= LAG:
                        emit_av(qt, ck - LAG)
                    for _ in range(2):
                        if mi < len(main_q):
                            main_q[mi]()
                            mi += 1
                        elif lazy_q:
                            lazy_q.pop(0)()
                while mi < len(main_q):
                    main_q[mi]()
                    mi += 1

                for ck in range(max(0, nch - LAG), nch):
                    pending_av.append((qt, ck))

            # ---------------- tail: qt = nqt-1, per-s pipelined ----------
            lq = nqt - 1
            while lazy_q:
                lazy_q.pop(0)()

            def tail_cproj_s(s):
                """at/linv extraction + c_proj for s-block s of last tile."""
                att = atts[lq]
                q0 = lq * QT
                ysb = yout_p.tile([128, C], F32, tag="ysb", name="ysb")
                for h in range(3):
                    at = at_p.tile([65, 128], BF16, tag="at", name="at_s")
                    nc.vector.tensor_copy(
                        at[:], att[:, h, s * 128:(s + 1) * 128]
                    )
                    lcol = lin_p.tile([128, 1], BF16, tag="lcol", name="lcol")
                    dma(lcol[:, :], at[64:65, :])
                    linv = lin_p.tile([128, 1], F32, tag="linv", name="linv")
                    nc.vector.reciprocal(linv[:], lcol[:])
                    yp = psy_p.tile([128, C], F32, tag="y", name="yp")
                    nc.tensor.matmul(
                        yp[:], at[0:64, :], wo[:, h, :],
                        start=True, stop=True,
                    )
                    if h == 0:
                        nc.vector.tensor_scalar(
                            out=ysb[:], in0=yp[:], scalar1=linv[:, 0:1],
                            scalar2=None, op0=MULT,
                        )
                    else:
                        nc.vector.scalar_tensor_tensor(
                            out=ysb[:], in0=yp[:], scalar=linv[:, 0:1],
                            in1=ysb[:], op0=MULT, op1=ADD,
                        )
                nc.gpsimd.dma_start(
                    out=y_d[q0 + s * 128:q0 + (s + 1) * 128, :], in_=ysb[:],
                )

            nch = 4 * nqt
            for (pqt, pck) in pending_av:
                emit_av(pqt, pck)
                # s-block s of the last q tile is final once the diagonal
                # chunk with qlo = 128*s has accumulated (ck = nch-4+s)
                if pck >= nch - 4:
                    tail_cproj_s(pck - (nch - 4))

    _split_excess_waits(nc)
    nc.finalize()
    return nc


_NC_CACHE = {}


def _get_nc(t=T):
    if t not in _NC_CACHE:
        _NC_CACHE[t] = build(t)
    return _NC_CACHE[t]


def _prep_core_inputs(x_b, w_attn, w_proj, hg, bf16):
    """Host-side shard prep for one core: batch x_b, head group hg (0/1)."""
    h0 = 3 * hg
    q = w_attn[:, 0:C]
    k = w_attn[:, C:2 * C]
    v = w_attn[:, 2 * C:3 * C]
    qcols = lambda h: q[:, h * D:(h + 1) * D]
    kcols = lambda h: k[:, h * D:(h + 1) * D]
    w_q01 = np.concatenate([qcols(h0), qcols(h0 + 1)], axis=1)      # [384,128]
    w_k01 = np.concatenate([kcols(h0), kcols(h0 + 1)], axis=1)
    w_q2 = qcols(h0 + 2)                                            # [384,64]
    w_k2 = kcols(h0 + 2)
    w_v = v[:, h0 * D:(h0 + 3) * D]                                 # [384,192]
    w_o = w_proj[h0 * D:(h0 + 3) * D, :]                            # [192,384]
    return {
        "xT16": np.ascontiguousarray(x_b.T, dtype=bf16),
        "w_q01": np.ascontiguousarray(w_q01.reshape(3, 128, 128), dtype=bf16),
        "w_k01": np.ascontiguousarray(w_k01.reshape(3, 128, 128), dtype=bf16),
        "w_q2": np.ascontiguousarray(w_q2.reshape(3, 128, 64), dtype=bf16),
        "w_k2": np.ascontiguousarray(w_k2.reshape(3, 128, 64), dtype=bf16),
        "w_v": np.ascontiguousarray(w_v.reshape(3, 128, 192), dtype=bf16),
        "w_o": np.ascontiguousarray(w_o.reshape(3, 64, 384), dtype=bf16),
    }


def _make_masks(bf16):
    f = np.arange(128)[None, :]
    p = np.arange(128)[:, None]
    return (f >= p).astype(np.float32).astype(bf16)


def kernel(x, w_attn, w_proj):
    import ml_dtypes
    bf16 = ml_dtypes.bfloat16

    x = np.asarray(x, dtype=np.float32)
    w_attn = np.asarray(w_attn, dtype=np.float32)
    w_proj = np.asarray(w_proj, dtype=np.float32)
    b, t, c = x.shape

    nc = _get_nc(t)
    masks = _make_masks(bf16)
    in_maps = []
    for core in range(8):
        im = _prep_core_inputs(x[core // 2], w_attn, w_proj, core % 2, bf16)
        im["masks"] = masks
        in_maps.append(im)

    res = run_bass_kernel_spmd(nc, in_maps, list(range(8)))
    out = np.empty((b, t, c), dtype=np.float32)
    for bb in range(b):
        out[bb] = res.results[2 * bb]["y"] + res.results[2 * bb + 1]["y"]
    return out


# revision 18
# speedup vs baseline: 1.0665x; 1.0539x over previous
"""Causal self-attention Trainium2 kernel (B=4, T=4096, C=384, H=6).

Sharding: 8 cores = 4 batches x 2 head-groups (3 heads each). Each core
computes y_partial = attn(x[b], heads hg) @ w_proj[rows of hg]; the host
sums the two partials per batch (the "all-reduce after c_proj" done on
host during unshard).

v2: chunk-granular pipeline. The Scalar engine (softmax exp, ~1 elem/
cycle/lane) is the bottleneck engine; everything else (S^T matmuls,
A@V accumulation, qkv projection, c_proj, output DMA) is interleaved
into the instruction streams so ACT stays busy continuously:
 - per 128-key chunk: S^T matmul pair -> exp -> (LAG chunks later) AV
 - AV is chunk-major across 3 PSUM accumulator banks (one per head)
 - c_proj(qt-1) + phase-A(qt+1) emitted as filler inside q-tile qt
 - causal-diagonal chunks restricted to the live q-range (saves exp
   elements and PE cycles); softmax denominator rides along as a 65th
   matmul row and is transposed via 4 tiny SBUF->SBUF DMAs.
"""

import numpy as np
from contextlib import ExitStack

import concourse.bass as bass
import concourse.tile as tile
from concourse import mybir
from concourse.bass_utils import run_bass_kernel_spmd
from concourse.vector_clock import ScopedClock

F32 = mybir.dt.float32
BF16 = mybir.dt.bfloat16
EXP = mybir.ActivationFunctionType.Exp
MULT = mybir.AluOpType.mult
ADD = mybir.AluOpType.add

B, T, C, H, D = 4, 4096, 384, 6, 64
HPC = 3            # heads per core
QT = 512           # q tile
KC = 128           # key chunk
SCALE = 1.0 / 8.0  # 1/sqrt(64)
LAG = 5            # chunks between exp and AV consumption


# ---------------------------------------------------------------------------
# Workaround: neuronxcc CoreV3 rejects >2 sem waits on the Tile tail drain.
# Split the drain's waits into individual sync-engine wait instructions.
def _drain_and_barrier_split(self, tick_clock, wait_clock):
    nc = self.nc
    drain_inst = nc.sync.drain()
    wait_clock.add_sem_waits(
        drain_inst.ins, ScopedClock({None: tick_clock.global_clock})
    )
    si = drain_inst.ins.sync_info
    if si is not None and si.on_wait and len(si.on_wait) > 1:
        waits = list(si.on_wait)
        si.on_wait = []
        allocated = {h.name: h for h in self.sems.allocated().values()}
        for w in waits:
            h = allocated.get(w.ant_name)
            assert h is not None, f"no sem handle for drain wait {w.ant_name}"
            assert w.wait_mode == "sem-ge-imm", w.wait_mode
            nc.sync.wait_ge(h, w.wait_value)
    nc.all_engine_barrier()
    assert self.sems is not None
    popped = nc._tile_sem_poison_stack.pop()
    assert popped is self._sem_poison
    nc.clear_and_free_semaphores(list(self.sems.allocated().values()))
    nc.all_engine_barrier()


tile.TileContext._drain_and_barrier = _drain_and_barrier_split


MAX_WAITS = 1  # CoreV3 per-instruction sem-wait capacity (S3_LW holds only 1)


def _split_excess_waits(nc):
    """Hoist sem waits beyond MAX_WAITS onto same-engine NOPs inserted
    directly before the over-limit instruction (waits are order-free)."""
    for fn in nc.m.functions:
        for bb in fn.blocks:
            insts = list(bb.instructions)
            out = []
            changed = False
            for inst in insts:
                si = inst.sync_info
                if si is not None and si.on_wait and len(si.on_wait) > MAX_WAITS:
                    waits = list(si.on_wait)
                    excess, keep = waits[:-MAX_WAITS], waits[-MAX_WAITS:]
                    si.on_wait = keep
                    inst.sync_info = si
                    for i in range(0, len(excess), MAX_WAITS):
                        nop = mybir.InstNoOp(
                            name=f"{inst.name}-waitsplit-{i}", ins=[], outs=[]
                        )
                        nop.engine = inst.engine
                        nop.sync_info = mybir.SyncInfo(
                            on_wait=excess[i:i + MAX_WAITS], on_update=[]
                        )
                        nc.register_instruction(nop)
                        out.append(nop)
                    changed = True
                out.append(inst)
            if changed:
                bb.instructions = out
# ---------------------------------------------------------------------------


def build(t=T):
    nqt = t // QT          # q tiles
    nkc = t // KC          # key chunks

    nc = bass.Bass()
    import itertools
    _dmaq = itertools.cycle([nc.sync, nc.gpsimd])

    def dma(out, in_):
        next(_dmaq).dma_start(out=out, in_=in_)
    x_d = nc.dram_tensor("xT16", [C, t], BF16, kind="ExternalInput")
    wq01_d = nc.dram_tensor("w_q01", [3, 128, 128], BF16, kind="ExternalInput")
    wk01_d = nc.dram_tensor("w_k01", [3, 128, 128], BF16, kind="ExternalInput")
    wq2_d = nc.dram_tensor("w_q2", [3, 128, 64], BF16, kind="ExternalInput")
    wk2_d = nc.dram_tensor("w_k2", [3, 128, 64], BF16, kind="ExternalInput")
    wv_d = nc.dram_tensor("w_v", [3, 128, 192], BF16, kind="ExternalInput")
    wo_d = nc.dram_tensor("w_o", [3, 64, 384], BF16, kind="ExternalInput")
    mask_d = nc.dram_tensor("masks", [128, 128], BF16, kind="ExternalInput")
    y_d = nc.dram_tensor("y", [t, C], F32, kind="ExternalOutput")

    with tile.TileContext(nc) as tc, ExitStack() as ctx:
        persist = ctx.enter_context(tc.tile_pool(name="persist", bufs=1))

        # weights / mask (DMAs emitted in the prologue, round-robin queues)
        wq01 = persist.tile([128, 3, 128], BF16)
        wk01 = persist.tile([128, 3, 128], BF16)
        wq2 = persist.tile([128, 3, 64], BF16)
        wk2 = persist.tile([128, 3, 64], BF16)
        wv = persist.tile([128, 3, 192], BF16)
        wo = persist.tile([64, 3, 384], BF16)
        m128 = persist.tile([128, 128], BF16)

        # persistent activations (bf16)
        qT01 = persist.tile([128, t], BF16)   # rows 0:64 h0 qT, 64:128 h1 qT
        kT01 = persist.tile([128, t], BF16)
        # head 2 q/k duplicated into both partition halves so chunk pairs
        # can run as concurrent row-group-packed matmuls
        qT2 = persist.tile([128, t], BF16)
        kT2 = persist.tile([128, t], BF16)
        vsb = persist.tile([128, nkc, 3, 65], BF16)  # [keys, chunk, head, d|one]
        nc.vector.memset(vsb[:, :, :, 64:65], 1.0)

        with (
            tc.tile_pool(name="xt", bufs=3) as xt_p,
            tc.tile_pool(name="ps", bufs=2, space="PSUM") as ps_p,        # 2x 2 banks
            tc.tile_pool(name="ps_att", bufs=1, space="PSUM") as psatt_p,  # 3 banks
            tc.tile_pool(name="ps_y", bufs=1, space="PSUM") as psy_p,      # 1 bank
            tc.tile_pool(name="pth01", bufs=10) as pth01_p,
            tc.tile_pool(name="pth2", bufs=6) as pth2_p,
            tc.tile_pool(name="at", bufs=6) as at_p,
            tc.tile_pool(name="lin", bufs=12) as lin_p,
            tc.tile_pool(name="yout", bufs=3) as yout_p,
        ):
            xts = {}        # tb -> xT tile
            ssxs = {}       # (qt, ck) -> psum S tile (h01)
            sscs = {}       # (qt, g) -> psum S tile (h2 pair)
            pth01s = {}     # (qt, ck) -> tile
            pth2s = {}      # (qt, g) -> tile
            atts = {}       # qt -> psum att tile [65, 3, 512]
            ats = {}        # qt -> [at_h tiles]
            linvs = {}      # qt -> [linv tiles]

            def load_x(tb):
                xT = xt_p.tile([128, 3, QT], BF16, tag="xt", name="xT")
                xts[tb] = xT
                for c in range(3):
                    dma(
                        xT[:, c, :],
                        x_d[c * 128:(c + 1) * 128, tb * QT:(tb + 1) * QT],
                    )

            def warm_burst(n=12):
                # The PE HAM clock gate only flips to K=8/8 (2.4 GHz) after a
                # fully-busy 4096-cycle window, and once warm it stays warm
                # through the pipeline's sub-microsecond gaps. Force the flip
                # with a dense dependency-free matmul burst; result unused.
                wb = ps_p.tile([128, 2, QT], F32, tag="ps", name="warmb")
                for _ in range(n):
                    nc.tensor.matmul(
                        wb[:, 0, 0:384], wq01[:, 0, :],
                        wq01[:, :, :].rearrange("p c m -> p (c m)"),
                        start=True, stop=True,
                    )

            def a_qk(tb, gi):
                w_sb, m, dst = (
                    (wq01, 128, qT01), (wk01, 128, kT01),
                    (wq2, 64, qT2), (wk2, 64, kT2),
                )[gi]
                xT = xts[tb]
                ps = ps_p.tile([128, QT], F32, tag="ps", name="psqk")
                for c in range(3):
                    nc.tensor.matmul(
                        ps[0:m, :], w_sb[:, c, 0:m], xT[:, c, :],
                        start=(c == 0), stop=(c == 2),
                    )
                nc.vector.tensor_copy(
                    dst[0:m, tb * QT:(tb + 1) * QT], ps[0:m, :]
                )

            def a_dup(tb):
                dma(
                    qT2[64:128, tb * QT:(tb + 1) * QT],
                    qT2[0:64, tb * QT:(tb + 1) * QT],
                )
                dma(
                    kT2[64:128, tb * QT:(tb + 1) * QT],
                    kT2[0:64, tb * QT:(tb + 1) * QT],
                )

            def a_v(tb, s):
                xT = xts[tb]
                psv = ps_p.tile([128, 3, 64], F32, tag="ps", name="psv")
                for c in range(3):
                    nc.tensor.matmul(
                        psv[:, :, :].rearrange("p h d -> p (h d)"),
                        xT[:, c, s * 128:(s + 1) * 128],
                        wv[:, c, :],
                        start=(c == 0), stop=(c == 2),
                    )
                nc.vector.tensor_copy(
                    vsb[:, tb * 4 + s, :, 0:64], psv[:, :, :]
                )

            def _qlo(qt, ck):
                return 128 * (ck - 4 * qt) if ck >= 4 * qt else 0

            def emit_S(qt, ck):
                """S^T matmuls for chunk ck (h01 pair; h2 pair on even ck).
                Emitted one chunk AHEAD of the exp so the PE computes S(ck+1)
                while ACT runs exp(ck)."""
                q0, q1 = qt * QT, (qt + 1) * QT
                qlo = _qlo(qt, ck)
                ssx = ps_p.tile([128, 2, QT], F32, tag="ps", name="ssx")
                ssxs[(qt, ck)] = ssx
                nc.tensor.matmul(
                    ssx[:, 0, qlo:],
                    kT01[0:64, ck * KC:(ck + 1) * KC],
                    qT01[0:64, q0 + qlo:q1],
                    start=True, stop=True, tile_position=(0, 0),
                )
                nc.tensor.matmul(
                    ssx[:, 1, qlo:],
                    kT01[64:128, ck * KC:(ck + 1) * KC],
                    qT01[64:128, q0 + qlo:q1],
                    start=True, stop=True, tile_position=(64, 0),
                )
                if ck % 2 == 0:
                    ck1 = ck + 1
                    qlo1 = _qlo(qt, ck1)
                    ssc = ps_p.tile([128, 2, QT], F32, tag="ps", name="ssc")
                    sscs[(qt, ck // 2)] = ssc
                    nc.tensor.matmul(
                        ssc[:, 0, qlo:],
                        kT2[0:64, ck * KC:(ck + 1) * KC],
                        qT2[0:64, q0 + qlo:q1],
                        start=True, stop=True, tile_position=(0, 0),
                    )
                    nc.tensor.matmul(
                        ssc[:, 1, qlo1:],
                        kT2[64:128, ck1 * KC:(ck1 + 1) * KC],
                        qT2[64:128, q0 + qlo1:q1],
                        start=True, stop=True, tile_position=(64, 0),
                    )

            def emit_exp(qt, ck):
                """exp + causal masks for chunk ck."""
                qlo = _qlo(qt, ck)
                diag = ck >= 4 * qt
                ssx = ssxs.pop((qt, ck))
                p01 = pth01_p.tile([128, 2, QT], BF16, tag="pth01", name="p01")
                pth01s[(qt, ck)] = p01
                nc.scalar.activation(
                    out=p01[:, :, qlo:], in_=ssx[:, :, qlo:],
                    func=EXP, scale=SCALE,
                )
                if ck % 2 == 0:
                    g = ck // 2
                    qlo1 = _qlo(qt, ck + 1)
                    ssc = sscs.pop((qt, g))
                    p2 = pth2_p.tile([128, 2, QT], BF16, tag="pth2", name="p2")
                    pth2s[(qt, g)] = p2
                    if qlo == qlo1:
                        nc.scalar.activation(
                            out=p2[:, :, qlo:], in_=ssc[:, :, qlo:],
                            func=EXP, scale=SCALE,
                        )
                    else:
                        nc.scalar.activation(
                            out=p2[:, 0, qlo:], in_=ssc[:, 0, qlo:],
                            func=EXP, scale=SCALE,
                        )
                        nc.scalar.activation(
                            out=p2[:, 1, qlo1:], in_=ssc[:, 1, qlo1:],
                            func=EXP, scale=SCALE,
                        )
                if diag:
                    w0, w1 = qlo, qlo + 128
                    for h in range(2):
                        nc.vector.tensor_tensor(
                            out=p01[:, h, w0:w1], in0=p01[:, h, w0:w1],
                            in1=m128[:], op=MULT,
                        )
                    p2t = pth2s[(qt, ck // 2)]
                    nc.vector.tensor_tensor(
                        out=p2t[:, ck % 2, w0:w1], in0=p2t[:, ck % 2, w0:w1],
                        in1=m128[:], op=MULT,
                    )

            def get_att(qt):
                if qt not in atts:
                    atts[qt] = psatt_p.tile(
                        [65, 3, QT], F32, tag="att", name="att"
                    )
                return atts[qt]

            def emit_av(qt, ck):
                nch = 4 * (qt + 1)
                att = get_att(qt)
                diag = ck >= 4 * qt
                qlo = 128 * (ck - 4 * qt) if diag else 0
                first, last = ck == 0, ck == nch - 1
                for h in range(3):
                    if h < 2:
                        rhs = pth01s[(qt, ck)][:, h, qlo:]
                    else:
                        rhs = pth2s[(qt, ck // 2)][:, ck % 2, qlo:]
                    nc.tensor.matmul(
                        att[:, h, qlo:], vsb[:, ck, h, :], rhs,
                        start=first, stop=last, skip_group_check=True,
                    )

            def emit_epilogue(qt):
                """att psum -> at (bf16, incl. denom row); l -> linv."""
                att = atts[qt]
                ats[qt] = []
                linvs[qt] = []
                for h in range(3):
                    at = at_p.tile([65, QT], BF16, tag="at", name="at")
                    ats[qt].append(at)
                    nc.vector.tensor_copy(at[:], att[:, h, :])
                    lcol = lin_p.tile([128, 4], BF16, tag="lcol", name="lcol")
                    for s in range(4):
                        dma(
                            lcol[:, s:s + 1],
                            at[64:65, s * 128:(s + 1) * 128],
                        )
                    linv = lin_p.tile([128, 4], F32, tag="linv", name="linv")
                    linvs[qt].append(linv)
                    nc.vector.reciprocal(linv[:], lcol[:])

            def emit_cproj_item(qt, s, h):
                """One head's c_proj matmul + normalize-accumulate for
                s-block s of q tile qt; DMA out after h==2."""
                q0 = qt * QT
                at = ats[qt][h]
                linv = linvs[qt][h]
                if h == 0:
                    ysb = yout_p.tile([128, C], F32, tag="ysb", name="ysb")
                    emit_cproj_item.ysb = ysb
                else:
                    ysb = emit_cproj_item.ysb
                yp = psy_p.tile([128, C], F32, tag="y", name="yp")
                nc.tensor.matmul(
                    yp[:], at[0:64, s * 128:(s + 1) * 128], wo[:, h, :],
                    start=True, stop=True,
                )
                sc = linv[:, s:s + 1]
                if h == 0:
                    nc.vector.tensor_scalar(
                        out=ysb[:], in0=yp[:], scalar1=sc,
                        scalar2=None, op0=MULT,
                    )
                else:
                    nc.vector.scalar_tensor_tensor(
                        out=ysb[:], in0=yp[:], scalar=sc, in1=ysb[:],
                        op0=MULT, op1=ADD,
                    )
                if h == 2:
                    nc.gpsimd.dma_start(
                        out=y_d[q0 + s * 128:q0 + (s + 1) * 128, :],
                        in_=ysb[:],
                    )

            # ---------------- prologue ----------------
            load_x(0)
            for c in range(3):
                dma(wq01[:, c, :], wq01_d[c])
                dma(wk01[:, c, :], wk01_d[c])
                dma(wq2[:, c, :], wq2_d[c])
                dma(wk2[:, c, :], wk2_d[c])
            load_x(1)
            for c in range(3):
                dma(wv[:, c, :], wv_d[c])
                dma(wo[:, c, :], wo_d[c])
            dma(m128[:], mask_d[:])
            warm_burst()
            for gi in range(4):
                a_qk(0, gi)
            a_dup(0)
            for s in range(4):
                a_v(0, s)

            # ---------------- main loop ----------------
            pending_av = []   # (qt, ck) AV triples carried into next qt
            lazy_q = []       # c_proj items; may straggle across qt bounds

            for qt in range(nqt):
                nch = 4 * (qt + 1)

                main_q = []
                if qt + 2 < nqt:
                    main_q.append(lambda tb=qt + 2: load_x(tb))
                carried, pending_av = pending_av, []
                for (pqt, pck) in carried:
                    main_q.append(lambda a=pqt, b=pck: emit_av(a, b))
                if qt >= 1:
                    main_q.append(lambda a=qt - 1: emit_epilogue(a))
                if qt + 1 < nqt:
                    for gi in range(4):
                        main_q.append(lambda tb=qt + 1, g=gi: a_qk(tb, g))
                    main_q.append(lambda tb=qt + 1: a_dup(tb))
                    for s in range(4):
                        main_q.append(lambda tb=qt + 1, b=s: a_v(tb, b))
                if qt >= 1:
                    for s in range(4):
                        for h in range(3):
                            lazy_q.append(
                                lambda a=qt - 1, b=s, c=h:
                                emit_cproj_item(a, b, c)
                            )

                mi = 0
                for ck in range(nch):
                    if ck == 0 and qt == 0:
                        emit_S(qt, 0)
                    if ck + 1 < nch:
                        emit_S(qt, ck + 1)
                    emit_exp(qt, ck)
                    if ck >= LAG:
                        emit_av(qt, ck - LAG)
                    for _ in range(2):
                        if mi < len(main_q):
                            main_q[mi]()
                            mi += 1
                        elif lazy_q:
                            lazy_q.pop(0)()
                while mi < len(main_q):
                    main_q[mi]()
                    mi += 1
                if qt + 1 < nqt:
                    emit_S(qt + 1, 0)   # prime next tile across the boundary

                for ck in range(max(0, nch - LAG), nch):
                    pending_av.append((qt, ck))

            # ---------------- tail: qt = nqt-1, per-s pipelined ----------
            lq = nqt - 1
            while lazy_q:
                lazy_q.pop(0)()

            def tail_cproj_s(s):
                """at/linv extraction + c_proj for s-block s of last tile."""
                att = atts[lq]
                q0 = lq * QT
                ysb = yout_p.tile([128, C], F32, tag="ysb", name="ysb")
                for h in range(3):
                    at = at_p.tile([65, 128], BF16, tag="at", name="at_s")
                    nc.vector.tensor_copy(
                        at[:], att[:, h, s * 128:(s + 1) * 128]
                    )
                    lcol = lin_p.tile([128, 1], BF16, tag="lcol", name="lcol")
                    dma(lcol[:, :], at[64:65, :])
                    linv = lin_p.tile([128, 1], F32, tag="linv", name="linv")
                    nc.vector.reciprocal(linv[:], lcol[:])
                    yp = psy_p.tile([128, C], F32, tag="y", name="yp")
                    nc.tensor.matmul(
                        yp[:], at[0:64, :], wo[:, h, :],
                        start=True, stop=True,
                    )
                    if h == 0:
                        nc.vector.tensor_scalar(
                            out=ysb[:], in0=yp[:], scalar1=linv[:, 0:1],
                            scalar2=None, op0=MULT,
                        )
                    else:
                        nc.vector.scalar_tensor_tensor(
                            out=ysb[:], in0=yp[:], scalar=linv[:, 0:1],
                            in1=ysb[:], op0=MULT, op1=ADD,
                        )
                nc.gpsimd.dma_start(
                    out=y_d[q0 + s * 128:q0 + (s + 1) * 128, :], in_=ysb[:],
                )

            nch = 4 * nqt
            for (pqt, pck) in pending_av:
                emit_av(pqt, pck)
                # s-block s of the last q tile is final once the diagonal
                # chunk with qlo = 128*s has accumulated (ck = nch-4+s)
                if pck >= nch - 4:
                    tail_cproj_s(pck - (nch - 4))

    _split_excess_waits(nc)
    nc.finalize()
    return nc


_NC_CACHE = {}


def _get_nc(t=T):
    if t not in _NC_CACHE:
        _NC_CACHE[t] = build(t)
    return _NC_CACHE[t]


def _prep_core_inputs(x_b, w_attn, w_proj, hg, bf16):
    """Host-side shard prep for one core: batch x_b, head group hg (0/1)."""
    h0 = 3 * hg
    q = w_attn[:, 0:C]
    k = w_attn[:, C:2 * C]
    v = w_attn[:, 2 * C:3 * C]
    qcols = lambda h: q[:, h * D:(h + 1) * D]
    kcols = lambda h: k[:, h * D:(h + 1) * D]
    w_q01 = np.concatenate([qcols(h0), qcols(h0 + 1)], axis=1)      # [384,128]
    w_k01 = np.concatenate([kcols(h0), kcols(h0 + 1)], axis=1)
    w_q2 = qcols(h0 + 2)                                            # [384,64]
    w_k2 = kcols(h0 + 2)
    w_v = v[:, h0 * D:(h0 + 3) * D]                                 # [384,192]
    w_o = w_proj[h0 * D:(h0 + 3) * D, :]                            # [192,384]
    return {
        "xT16": np.ascontiguousarray(x_b.T, dtype=bf16),
        "w_q01": np.ascontiguousarray(w_q01.reshape(3, 128, 128), dtype=bf16),
        "w_k01": np.ascontiguousarray(w_k01.reshape(3, 128, 128), dtype=bf16),
        "w_q2": np.ascontiguousarray(w_q2.reshape(3, 128, 64), dtype=bf16),
        "w_k2": np.ascontiguousarray(w_k2.reshape(3, 128, 64), dtype=bf16),
        "w_v": np.ascontiguousarray(w_v.reshape(3, 128, 192), dtype=bf16),
        "w_o": np.ascontiguousarray(w_o.reshape(3, 64, 384), dtype=bf16),
    }


def _make_masks(bf16):
    f = np.arange(128)[None, :]
    p = np.arange(128)[:, None]
    return (f >= p).astype(np.float32).astype(bf16)


def kernel(x, w_attn, w_proj):
    import ml_dtypes
    bf16 = ml_dtypes.bfloat16

    x = np.asarray(x, dtype=np.float32)
    w_attn = np.asarray(w_attn, dtype=np.float32)
    w_proj = np.asarray(w_proj, dtype=np.float32)
    b, t, c = x.shape

    nc = _get_nc(t)
    masks = _make_masks(bf16)
    in_maps = []
    for core in range(8):
        im = _prep_core_inputs(x[core // 2], w_attn, w_proj, core % 2, bf16)
        im["masks"] = masks
        in_maps.append(im)

    res = run_bass_kernel_spmd(nc, in_maps, list(range(8)))
    out = np.empty((b, t, c), dtype=np.float32)
    for bb in range(b):
        out[bb] = res.results[2 * bb]["y"] + res.results[2 * bb + 1]["y"]
    return out
